# revision 5
# baseline (speedup 1.0000x reference)
"""Multi-head causal attention + RoPE, sharded over 8 TRN2 NeuronCores.

Sharding: core c -> batch b = c//4, head-group g = c%4 (4 of 16 heads).
Each core computes a partial output x[b] @ (its heads' slice); host sums
the 4 group partials per batch.

Device algorithm (per core, 4 heads, S=2048, D=1024, HD=64):
  - host supplies x[b]^T so all matmuls use natural K-on-partition layouts
  - QK projections produce rope-split components (even/odd pair halves)
    directly via host-permuted weight columns; RoPE applied with full
    128-partition DVE ops (4 heads x 32 components stacked)
  - scores computed transposed [sk, sq] (lhsT = k, rhs = q), exp on ACT
    without max-subtraction (scores are O(1) here), block-causal skipping
  - ctx^T = [v | 1]^T @ exp gives both context and softmax denominators
    in one accumulation (ones-column trick); normalize when copying into
    the out-projection lhsT buffer
  - out = ctxN^T.T @ Wo rows for this head group
Matmuls run as float32r (full-rate fp32 mode, ~1e-4 rel precision).
"""

import os
import sys

import numpy as np

for _p in ("/opt/trn_rl_repo", "/root/.axon_site/_ro/trn_rl_repo"):
    if os.path.isdir(_p) and _p not in sys.path:
        sys.path.append(_p)

import concourse.bass as bass  # noqa: E402
import concourse.mybir as mybir  # noqa: E402
import concourse.tile as tile  # noqa: E402
from concourse import bacc  # noqa: E402
from concourse.bass import ts  # noqa: E402
from concourse.bass_utils import run_bass_kernel_spmd  # noqa: E402

B, S, D = 2, 2048, 1024
HEADS, HD = 16, 64
G = 4                      # head groups == cores per batch element
HPC = HEADS // G           # heads per core
NCOL = HPC * HD            # 256 projection cols per core
KCH = D // 128             # K chunks
MCH = S // 128             # sk chunks
TCH = S // 512             # sq 512-blocks
F32 = mybir.dt.float32
F32R = mybir.dt.float32r
AF = mybir.ActivationFunctionType

TRACE = False
TRACE_DIR = None
LAST_EXEC_NS = None
_CACHE = {}


def _build():
    nc = bacc.Bacc("TRN2")
    xT_d = nc.dram_tensor("xT", (D, S), F32R, kind="ExternalInput")
    wqe_d = nc.dram_tensor("wqe", (D, 128), F32R, kind="ExternalInput")
    wqo_d = nc.dram_tensor("wqo", (D, 128), F32R, kind="ExternalInput")
    wke_d = nc.dram_tensor("wke", (D, 128), F32R, kind="ExternalInput")
    wko_d = nc.dram_tensor("wko", (D, 128), F32R, kind="ExternalInput")
    wv_d = nc.dram_tensor("wv", (D, NCOL), F32R, kind="ExternalInput")
    wo_d = nc.dram_tensor("wo", (NCOL, D), F32R, kind="ExternalInput")
    cos_d = nc.dram_tensor("cosT", (128, S), F32, kind="ExternalInput")
    sin_d = nc.dram_tensor("sinT", (128, S), F32, kind="ExternalInput")
    mask_d = nc.dram_tensor("mask", (128, 4 * 512), F32, kind="ExternalInput")
    ones1_d = nc.dram_tensor("ones1", (1, 64), F32R, kind="ExternalInput")
    vones_d = nc.dram_tensor("vones", (128, MCH * HPC), F32R,
                             kind="ExternalInput")
    out_d = nc.dram_tensor("out", (S, D), F32, kind="ExternalOutput")

    def mmr(ps, lhsT, rhs, **kw):
        nc.tensor.matmul(ps, lhsT, rhs, **kw)

    with tile.TileContext(nc) as tc:
        with tc.tile_pool(name="pp", bufs=1) as pp, \
             tc.tile_pool(name="tmp3", bufs=3) as tmp3, \
             tc.tile_pool(name="tmp2", bufs=2) as tmp2:
            rqe = pp.tile([128, S], F32R)
            rqo = pp.tile([128, S], F32R)
            rke = pp.tile([128, S], F32R)
            rko = pp.tile([128, S], F32R)
            v_aug = pp.tile([128, MCH, HPC, HD + 1], F32R)
            mask_sb = pp.tile([128, 4 * 512], F32)
            ones1 = pp.tile([1, 64], F32R)
            nc.sync.dma_start(ones1[:], ones1_d[:])
            nc.sync.dma_start(
                v_aug[:, :, :, HD],
                vones_d.rearrange("p (m h) -> p m h", m=MCH))
            nc.sync.dma_start(mask_sb[:], mask_d[:])

            with tc.tile_pool(name="pA", bufs=1) as pA, \
                 tc.tile_pool(name="psQ", bufs=8, space="PSUM") as psQ:
                w_sb = {}
                for nm, d_t, w in (("wqe", wqe_d, 128), ("wqo", wqo_d, 128),
                                   ("wke", wke_d, 128), ("wko", wko_d, 128),
                                   ("wv", wv_d, NCOL)):
                    t_sb = pA.tile([128, KCH, w], F32R, tag=nm)
                    nc.sync.dma_start(
                        t_sb[:], d_t.rearrange("(ko p) m -> p ko m", p=128))
                    w_sb[nm] = t_sb
                cos_sb = pA.tile([128, S], F32)
                sin_sb = pA.tile([128, S], F32)
                nc.sync.dma_start(cos_sb[:], cos_d[:])
                nc.sync.dma_start(sin_sb[:], sin_d[:])
                xt = pA.tile([128, KCH, S], F32R)
                for k in range(KCH):
                    nc.sync.dma_start(xt[:, k, :], xT_d[ts(k, 128), :])

                def rope_pair(e_ps, o_ps, re, ro, sl):
                    t1 = tmp3.tile([128, 512], F32, tag="ropet")
                    t2 = tmp3.tile([128, 512], F32, tag="ropeu")
                    nc.vector.tensor_mul(t1[:], e_ps[:], cos_sb[:, sl])
                    nc.vector.tensor_mul(t2[:], o_ps[:], sin_sb[:, sl])
                    nc.vector.tensor_sub(re[:, sl], t1[:], t2[:])
                    t3 = tmp3.tile([128, 512], F32, tag="ropet")
                    t4 = tmp3.tile([128, 512], F32, tag="ropeu")
                    nc.vector.tensor_mul(t3[:], e_ps[:], sin_sb[:, sl])
                    nc.vector.tensor_mul(t4[:], o_ps[:], cos_sb[:, sl])
                    nc.vector.tensor_add(ro[:, sl], t3[:], t4[:])

                for we, wod, re, ro in ((w_sb["wqe"], w_sb["wqo"], rqe, rqo),
                                        (w_sb["wke"], w_sb["wko"], rke, rko)):
                    e_pss = [psQ.tile([128, 512], F32, tag="ps",
                                      name=f"eps{n}")
                             for n in range(TCH)]
                    o_pss = [psQ.tile([128, 512], F32, tag="ps",
                                      name=f"ops{n}")
                             for n in range(TCH)]
                    for k in range(KCH):
                        for n in range(TCH):
                            mmr(e_pss[n][:], we[:, k, :], xt[:, k, ts(n, 512)],
                                start=(k == 0), stop=(k == KCH - 1))
                            mmr(o_pss[n][:], wod[:, k, :], xt[:, k, ts(n, 512)],
                                start=(k == 0), stop=(k == KCH - 1))
                    for n in range(TCH):
                        rope_pair(e_pss[n], o_pss[n], re, ro, ts(n, 512))

                for m in range(MCH):
                    v_ps = psQ.tile([128, NCOL], F32, tag="ps")
                    for k in range(KCH):
                        mmr(v_ps[:], xt[:, k, ts(m, 128)], w_sb["wv"][:, k, :],
                            start=(k == 0), stop=(k == KCH - 1))
                    nc.vector.tensor_copy(
                        v_aug[:, m, :, 0:HD],
                        v_ps.rearrange("p (h d) -> p h d", h=HPC))

            with tc.tile_pool(name="pF", bufs=1) as pF, \
                 tc.tile_pool(name="psS", bufs=4, space="PSUM") as psS, \
                 tc.tile_pool(name="psC", bufs=4, space="PSUM") as psC:
                wo_sb = pF.tile([128, 2, D], F32R)
                nc.sync.dma_start(
                    wo_sb[:], wo_d.rearrange("(j p) n -> p j n", p=128))
                ct = pF.tile([128, 2, S], F32R)

                for t in range(TCH):
                    ctx_ps = [psC.tile([65, 512], F32, tag="ctx",
                                       name=f"ctx{h}")
                              for h in range(HPC)]
                    for c in range(4 * t + 4):
                        sc = []
                        for h in range(HPC):
                            sc_ps = psS.tile([128, 512], F32, tag="sc",
                                             name=f"sc{h}")
                            sc.append(sc_ps)
                            mmr(sc_ps[:], rke[32 * h:32 * h + 32, ts(c, 128)],
                                rqe[32 * h:32 * h + 32, ts(t, 512)],
                                start=True, stop=False,
                                tile_position=(32 * h, 0))
                        for h in range(HPC):
                            mmr(sc[h][:], rko[32 * h:32 * h + 32, ts(c, 128)],
                                rqo[32 * h:32 * h + 32, ts(t, 512)],
                                start=False, stop=True,
                                tile_position=(32 * h, 0))
                        for h in range(HPC):
                            e_sb = tmp3.tile([128, 512], F32R, tag="exp")
                            nc.scalar.activation(e_sb[:], sc[h][:], AF.Exp)
                            if c // 4 == t:
                                nc.gpsimd.tensor_mul(
                                    e_sb[:], e_sb[:],
                                    mask_sb[:, ts(c % 4, 512)])
                            mmr(ctx_ps[h][:], v_aug[:, c, h, :], e_sb[:],
                                start=(c == 0), stop=(c == 4 * t + 3))
                    for h in range(HPC):
                        d_sb = tmp2.tile([1, 512], F32R, tag="dsb")
                        nc.vector.tensor_copy(d_sb[:], ctx_ps[h][64:65, :])
                        rb_ps = psS.tile([128, 512], F32, tag="sc")
                        mmr(rb_ps[0:64, :], ones1[:], d_sb[:],
                            start=True, stop=True)
                        rec = tmp2.tile([64, 512], F32, tag="rec")
                        nc.vector.reciprocal(rec[:], rb_ps[0:64, :])
                        nc.vector.tensor_mul(
                            ct[64 * (h % 2):64 * (h % 2) + 64, h // 2,
                               ts(t, 512)],
                            ctx_ps[h][0:64, :], rec[:])

                for m in range(MCH):
                    for j2 in range(2):
                        o_ps = psS.tile([128, 512], F32, tag="sc")
                        mmr(o_ps[:], ct[:, 0, ts(m, 128)],
                            wo_sb[:, 0, ts(j2, 512)], start=True, stop=False)
                        mmr(o_ps[:], ct[:, 1, ts(m, 128)],
                            wo_sb[:, 1, ts(j2, 512)], start=False, stop=True)
                        o_sb = tmp2.tile([128, 512], F32, tag="osb")
                        nc.scalar.copy(o_sb[:], o_ps[:])
                        nc.sync.dma_start(
                            out_d[ts(m, 128), ts(j2, 512)], o_sb[:])
    nc.compile()
    return nc


def _host_tables():
    half = HD // 2
    inv_freq = (1.0 / (10000.0 ** (np.arange(half, dtype=np.float32) / half)))
    angles = (np.arange(S, dtype=np.float32)[:, None]
              * inv_freq[None, :].astype(np.float32))
    cosT = np.tile(np.cos(angles).T.astype(np.float32), (HPC, 1))
    sinT = np.tile(np.sin(angles).T.astype(np.float32), (HPC, 1))
    mask = np.zeros((128, 4 * 512), dtype=np.float32)
    i_idx = np.arange(128)[:, None]
    j_idx = np.arange(512)[None, :]
    for m in range(4):
        mask[:, m * 512:(m + 1) * 512] = (
            (j_idx - 128 * m) >= i_idx).astype(np.float32)
    return np.ascontiguousarray(cosT), np.ascontiguousarray(sinT), mask


def kernel(x, Wq, Wk, Wv, Wo):
    global LAST_EXEC_NS
    x = np.asarray(x, dtype=np.float32)
    Wq = np.asarray(Wq, dtype=np.float32)
    Wk = np.asarray(Wk, dtype=np.float32)
    Wv = np.asarray(Wv, dtype=np.float32)
    Wo = np.asarray(Wo, dtype=np.float32)

    if "nc" not in _CACHE:
        _CACHE["nc"] = _build()
    nc = _CACHE["nc"]
    cosT, sinT, mask = _host_tables()

    in_maps = []
    for c in range(8):
        b, g = c // 4, c % 4
        cols = slice(g * NCOL, (g + 1) * NCOL)
        wq_g = Wq[:, cols].reshape(D, HPC, HD // 2, 2)
        wk_g = Wk[:, cols].reshape(D, HPC, HD // 2, 2)
        in_maps.append({
            "xT": np.ascontiguousarray(x[b].T),
            "wqe": np.ascontiguousarray(
                wq_g[..., 0].reshape(D, 128) * 0.125),
            "wqo": np.ascontiguousarray(
                wq_g[..., 1].reshape(D, 128) * 0.125),
            "wke": np.ascontiguousarray(wk_g[..., 0].reshape(D, 128)),
            "wko": np.ascontiguousarray(wk_g[..., 1].reshape(D, 128)),
            "wv": np.ascontiguousarray(Wv[:, cols]),
            "wo": np.ascontiguousarray(Wo[cols, :]),
            "cosT": cosT,
            "ones1": np.ones((1, 64), dtype=np.float32),
            "vones": np.ones((128, MCH * HPC), dtype=np.float32),
            "sinT": sinT,
            "mask": mask,
        })

    kw = {}
    if TRACE and TRACE_DIR:
        os.makedirs(TRACE_DIR, exist_ok=True)
        kw["tmpdir"] = TRACE_DIR
    res = run_bass_kernel_spmd(nc, in_maps, core_ids=list(range(8)),
                               trace=TRACE, **kw)
    LAST_EXEC_NS = res.exec_time_ns
    parts = [res.results[c]["out"] for c in range(8)]
    out = np.empty((B, S, D), dtype=np.float32)
    for b in range(B):
        out[b] = parts[4 * b] + parts[4 * b + 1] + parts[4 * b + 2] \
            + parts[4 * b + 3]
    return out


# revision 6
# speedup vs baseline: 1.0544x; 1.0544x over previous
"""Multi-head causal attention + RoPE, sharded over 8 TRN2 NeuronCores.

Sharding: core c -> batch b = c//4, head-group g = c%4 (4 of 16 heads).
Each core computes a partial output x[b] @ (its heads' slice); host sums
the 4 group partials per batch.

Device algorithm (per core, 4 heads, S=2048, D=1024, HD=64):
  - host supplies x[b]^T so all matmuls use natural K-on-partition layouts
  - QK projections produce rope-split components (even/odd pair halves)
    directly via host-permuted weight columns; RoPE applied with full
    128-partition DVE ops (4 heads x 32 components stacked)
  - scores computed transposed [sk, sq] (lhsT = k, rhs = q), exp on ACT
    without max-subtraction (scores are O(1) here), block-causal skipping
  - ctx^T = [v | 1]^T @ exp gives both context and softmax denominators
    in one accumulation (ones-column trick); normalize when copying into
    the out-projection lhsT buffer
  - out = ctxN^T.T @ Wo rows for this head group
Matmuls run as float32r (full-rate fp32 mode, ~1e-4 rel precision).
"""

import os
import sys

import numpy as np

for _p in ("/opt/trn_rl_repo", "/root/.axon_site/_ro/trn_rl_repo"):
    if os.path.isdir(_p) and _p not in sys.path:
        sys.path.append(_p)

import concourse.bass as bass  # noqa: E402
import concourse.mybir as mybir  # noqa: E402
import concourse.tile as tile  # noqa: E402
from concourse import bacc  # noqa: E402
from concourse.bass import ts  # noqa: E402
from concourse.bass_utils import run_bass_kernel_spmd  # noqa: E402

B, S, D = 2, 2048, 1024
HEADS, HD = 16, 64
G = 4                      # head groups == cores per batch element
HPC = HEADS // G           # heads per core
NCOL = HPC * HD            # 256 projection cols per core
KCH = D // 128             # K chunks
MCH = S // 128             # sk chunks
TCH = S // 512             # sq 512-blocks
F32 = mybir.dt.float32
F32R = mybir.dt.float32r
BF16 = mybir.dt.bfloat16
AF = mybir.ActivationFunctionType

TRACE = False
TRACE_DIR = None
LAST_EXEC_NS = None
_CACHE = {}


def _build():
    nc = bacc.Bacc("TRN2")
    xT_d = nc.dram_tensor("xT", (D, S), F32R, kind="ExternalInput")
    wqe_d = nc.dram_tensor("wqe", (D, 128), F32R, kind="ExternalInput")
    wqo_d = nc.dram_tensor("wqo", (D, 128), F32R, kind="ExternalInput")
    wke_d = nc.dram_tensor("wke", (D, 128), F32R, kind="ExternalInput")
    wko_d = nc.dram_tensor("wko", (D, 128), F32R, kind="ExternalInput")
    wv_d = nc.dram_tensor("wv", (D, NCOL), F32R, kind="ExternalInput")
    wo_d = nc.dram_tensor("wo", (NCOL, D), F32R, kind="ExternalInput")
    cos_d = nc.dram_tensor("cosT", (128, S), F32, kind="ExternalInput")
    sin_d = nc.dram_tensor("sinT", (128, S), F32, kind="ExternalInput")
    mask_d = nc.dram_tensor("mask", (128, 4 * 512), F32, kind="ExternalInput")
    ones1_d = nc.dram_tensor("ones1", (1, 64), F32R, kind="ExternalInput")
    vones_d = nc.dram_tensor("vones", (128, MCH * HPC), F32R,
                             kind="ExternalInput")
    out_d = nc.dram_tensor("out", (S, D), F32, kind="ExternalOutput")

    def mmr(ps, lhsT, rhs, **kw):
        nc.tensor.matmul(ps, lhsT, rhs, **kw)

    with tile.TileContext(nc) as tc:
        with tc.tile_pool(name="pp", bufs=1) as pp, \
             tc.tile_pool(name="tmp3", bufs=3) as tmp3, \
             tc.tile_pool(name="tmp2", bufs=2) as tmp2:
            rqe = pp.tile([128, S], BF16)
            rqo = pp.tile([128, S], BF16)
            rke = pp.tile([128, S], BF16)
            rko = pp.tile([128, S], BF16)
            v_aug = pp.tile([128, MCH, HPC, HD + 1], F32R)
            mask_sb = pp.tile([128, 4 * 512], F32)
            ones1 = pp.tile([1, 64], F32R)
            nc.sync.dma_start(ones1[:], ones1_d[:])
            nc.sync.dma_start(
                v_aug[:, :, :, HD],
                vones_d.rearrange("p (m h) -> p m h", m=MCH))
            nc.sync.dma_start(mask_sb[:], mask_d[:])

            with tc.tile_pool(name="pA", bufs=1) as pA, \
                 tc.tile_pool(name="psQ", bufs=8, space="PSUM") as psQ:
                w_sb = {}
                for nm, d_t, w in (("wqe", wqe_d, 128), ("wqo", wqo_d, 128),
                                   ("wke", wke_d, 128), ("wko", wko_d, 128),
                                   ("wv", wv_d, NCOL)):
                    t_sb = pA.tile([128, KCH, w], F32R, tag=nm)
                    nc.sync.dma_start(
                        t_sb[:], d_t.rearrange("(ko p) m -> p ko m", p=128))
                    w_sb[nm] = t_sb
                cos_sb = pA.tile([128, S], F32)
                sin_sb = pA.tile([128, S], F32)
                nc.sync.dma_start(cos_sb[:], cos_d[:])
                nc.sync.dma_start(sin_sb[:], sin_d[:])
                xt = pA.tile([128, KCH, S], F32R)
                for k in range(KCH):
                    nc.sync.dma_start(xt[:, k, :], xT_d[ts(k, 128), :])

                def rope_pair(e_ps, o_ps, re, ro, sl):
                    t1 = tmp3.tile([128, 512], F32, tag="ropet")
                    t2 = tmp3.tile([128, 512], F32, tag="ropeu")
                    nc.vector.tensor_mul(t1[:], e_ps[:], cos_sb[:, sl])
                    nc.vector.tensor_mul(t2[:], o_ps[:], sin_sb[:, sl])
                    nc.vector.tensor_sub(re[:, sl], t1[:], t2[:])
                    t3 = tmp3.tile([128, 512], F32, tag="ropet")
                    t4 = tmp3.tile([128, 512], F32, tag="ropeu")
                    nc.vector.tensor_mul(t3[:], e_ps[:], sin_sb[:, sl])
                    nc.vector.tensor_mul(t4[:], o_ps[:], cos_sb[:, sl])
                    nc.vector.tensor_add(ro[:, sl], t3[:], t4[:])

                for we, wod, re, ro in ((w_sb["wqe"], w_sb["wqo"], rqe, rqo),
                                        (w_sb["wke"], w_sb["wko"], rke, rko)):
                    e_pss = [psQ.tile([128, 512], F32, tag="ps",
                                      name=f"eps{n}")
                             for n in range(TCH)]
                    o_pss = [psQ.tile([128, 512], F32, tag="ps",
                                      name=f"ops{n}")
                             for n in range(TCH)]
                    for k in range(KCH):
                        for n in range(TCH):
                            mmr(e_pss[n][:], we[:, k, :], xt[:, k, ts(n, 512)],
                                start=(k == 0), stop=(k == KCH - 1))
                            mmr(o_pss[n][:], wod[:, k, :], xt[:, k, ts(n, 512)],
                                start=(k == 0), stop=(k == KCH - 1))
                    for n in range(TCH):
                        rope_pair(e_pss[n], o_pss[n], re, ro, ts(n, 512))

                for m in range(MCH):
                    v_ps = psQ.tile([128, NCOL], F32, tag="ps")
                    for k in range(KCH):
                        mmr(v_ps[:], xt[:, k, ts(m, 128)], w_sb["wv"][:, k, :],
                            start=(k == 0), stop=(k == KCH - 1))
                    nc.vector.tensor_copy(
                        v_aug[:, m, :, 0:HD],
                        v_ps.rearrange("p (h d) -> p h d", h=HPC))

            with tc.tile_pool(name="pF", bufs=1) as pF, \
                 tc.tile_pool(name="psS", bufs=6, space="PSUM") as psS, \
                 tc.tile_pool(name="psC", bufs=2, space="PSUM") as psC:
                wo_sb = pF.tile([128, 2, D], F32R)
                nc.sync.dma_start(
                    wo_sb[:], wo_d.rearrange("(j p) n -> p j n", p=128))
                ct = pF.tile([128, 2, S], F32R)

                for t in range(TCH):
                    for pair in range(2):
                        hs = (2 * pair, 2 * pair + 1)
                        ctx_ps = {h: psC.tile([65, 512], F32, tag="ctx",
                                              name=f"ctx{h}")
                                  for h in hs}
                        for c in range(4 * t + 4):
                            sc = {}
                            for h in hs:
                                sc[h] = psS.tile([128, 512], F32, tag="sc",
                                                 name=f"sc{h}")
                                nc.tensor.matmul(
                                    sc[h][:],
                                    rke[32 * h:32 * h + 32, ts(c, 128)],
                                    rqe[32 * h:32 * h + 32, ts(t, 512)],
                                    start=True, stop=False,
                                    tile_position=(32 * h, 0))
                            for h in hs:
                                nc.tensor.matmul(
                                    sc[h][:],
                                    rko[32 * h:32 * h + 32, ts(c, 128)],
                                    rqo[32 * h:32 * h + 32, ts(t, 512)],
                                    start=False, stop=True,
                                    tile_position=(32 * h, 0))
                            for h in hs:
                                e_sb = tmp3.tile([128, 512], F32R, tag="exp")
                                nc.scalar.activation(e_sb[:], sc[h][:], AF.Exp)
                                if c // 4 == t:
                                    nc.gpsimd.tensor_mul(
                                        e_sb[:], e_sb[:],
                                        mask_sb[:, ts(c % 4, 512)])
                                mmr(ctx_ps[h][:], v_aug[:, c, h, :], e_sb[:],
                                    start=(c == 0), stop=(c == 4 * t + 3))
                        for h in hs:
                            d_sb = tmp2.tile([1, 512], F32R, tag="dsb")
                            nc.vector.tensor_copy(d_sb[:], ctx_ps[h][64:65, :])
                            rb_ps = psS.tile([128, 512], F32, tag="sc")
                            mmr(rb_ps[0:64, :], ones1[:], d_sb[:],
                                start=True, stop=True)
                            rec = tmp2.tile([64, 512], F32, tag="rec")
                            nc.vector.reciprocal_approx_fast(
                                rec[:], rb_ps[0:64, :])
                            nc.vector.tensor_mul(
                                ct[64 * (h % 2):64 * (h % 2) + 64, h // 2,
                                   ts(t, 512)],
                                ctx_ps[h][0:64, :], rec[:])
                    for m4 in range(4):
                        m = 4 * t + m4
                        for j2 in range(2):
                            o_ps = psS.tile([128, 512], F32, tag="sc")
                            mmr(o_ps[:], ct[:, 0, ts(m, 128)],
                                wo_sb[:, 0, ts(j2, 512)],
                                start=True, stop=False)
                            mmr(o_ps[:], ct[:, 1, ts(m, 128)],
                                wo_sb[:, 1, ts(j2, 512)],
                                start=False, stop=True)
                            o_sb = tmp2.tile([128, 512], F32, tag="osb")
                            nc.vector.tensor_copy(o_sb[:], o_ps[:])
                            nc.sync.dma_start(
                                out_d[ts(m, 128), ts(j2, 512)], o_sb[:])
    nc.compile()
    return nc


def _host_tables():
    half = HD // 2
    inv_freq = (1.0 / (10000.0 ** (np.arange(half, dtype=np.float32) / half)))
    angles = (np.arange(S, dtype=np.float32)[:, None]
              * inv_freq[None, :].astype(np.float32))
    cosT = np.tile(np.cos(angles).T.astype(np.float32), (HPC, 1))
    sinT = np.tile(np.sin(angles).T.astype(np.float32), (HPC, 1))
    mask = np.zeros((128, 4 * 512), dtype=np.float32)
    i_idx = np.arange(128)[:, None]
    j_idx = np.arange(512)[None, :]
    for m in range(4):
        mask[:, m * 512:(m + 1) * 512] = (
            (j_idx - 128 * m) >= i_idx).astype(np.float32)
    return np.ascontiguousarray(cosT), np.ascontiguousarray(sinT), mask


def kernel(x, Wq, Wk, Wv, Wo):
    global LAST_EXEC_NS
    x = np.asarray(x, dtype=np.float32)
    Wq = np.asarray(Wq, dtype=np.float32)
    Wk = np.asarray(Wk, dtype=np.float32)
    Wv = np.asarray(Wv, dtype=np.float32)
    Wo = np.asarray(Wo, dtype=np.float32)

    if "nc" not in _CACHE:
        _CACHE["nc"] = _build()
    nc = _CACHE["nc"]
    cosT, sinT, mask = _host_tables()

    in_maps = []
    for c in range(8):
        b, g = c // 4, c % 4
        cols = slice(g * NCOL, (g + 1) * NCOL)
        wq_g = Wq[:, cols].reshape(D, HPC, HD // 2, 2)
        wk_g = Wk[:, cols].reshape(D, HPC, HD // 2, 2)
        in_maps.append({
            "xT": np.ascontiguousarray(x[b].T),
            "wqe": np.ascontiguousarray(
                wq_g[..., 0].reshape(D, 128) * 0.125),
            "wqo": np.ascontiguousarray(
                wq_g[..., 1].reshape(D, 128) * 0.125),
            "wke": np.ascontiguousarray(wk_g[..., 0].reshape(D, 128)),
            "wko": np.ascontiguousarray(wk_g[..., 1].reshape(D, 128)),
            "wv": np.ascontiguousarray(Wv[:, cols]),
            "wo": np.ascontiguousarray(Wo[cols, :]),
            "cosT": cosT,
            "ones1": np.ones((1, 64), dtype=np.float32),
            "vones": np.ones((128, MCH * HPC), dtype=np.float32),
            "sinT": sinT,
            "mask": mask,
        })

    kw = {}
    if TRACE and TRACE_DIR:
        os.makedirs(TRACE_DIR, exist_ok=True)
        kw["tmpdir"] = TRACE_DIR
    res = run_bass_kernel_spmd(nc, in_maps, core_ids=list(range(8)),
                               trace=TRACE, **kw)
    LAST_EXEC_NS = res.exec_time_ns
    parts = [res.results[c]["out"] for c in range(8)]
    out = np.empty((B, S, D), dtype=np.float32)
    for b in range(B):
        out[b] = parts[4 * b] + parts[4 * b + 1] + parts[4 * b + 2] \
            + parts[4 * b + 3]
    return out


# revision 7
# speedup vs baseline: 1.1118x; 1.0545x over previous
"""Multi-head causal attention + RoPE, sharded over 8 TRN2 NeuronCores.

Sharding: core c -> batch b = c//4, head-group g = c%4 (4 of 16 heads).
Each core computes a partial output x[b] @ (its heads' slice); host sums
the 4 group partials per batch.

Device algorithm (per core, 4 heads, S=2048, D=1024, HD=64):
  - host supplies x[b]^T so all matmuls use natural K-on-partition layouts
  - QK projections produce rope-split components (even/odd pair halves)
    directly via host-permuted weight columns; RoPE applied with full
    128-partition DVE ops (4 heads x 32 components stacked), output bf16
  - scores computed transposed [sk, sq] (lhsT = roped k, rhs = roped q),
    two sk-chunks batched per [128,1024] PSUM tile so one ACT exp call
    covers both (amortizes the ~352-cycle ACT pipeline fill); no
    max-subtraction (scores are O(1) here); block-causal skipping with a
    [128,128] triangle mask multiply on DVE for diagonal chunks
  - ctx^T = [v | 1]^T @ exp gives context and softmax denominators in one
    PSUM accumulation (ones-column trick); denominators broadcast across
    partitions via a K=1 matmul, reciprocal via the fast approx DVE op
  - out = ctxN^T.T @ Wo rows for this head group, folded into the t-loop
FAST=False: projections + exp/v path in float32r (~1e-3 total rel err).
FAST=True: those in bf16 (~2.5e-3 rel err, less PE time).
"""

import os
import sys

import numpy as np

for _p in ("/opt/trn_rl_repo", "/root/.axon_site/_ro/trn_rl_repo"):
    if os.path.isdir(_p) and _p not in sys.path:
        sys.path.append(_p)

import ml_dtypes  # noqa: E402

import concourse.bass as bass  # noqa: E402
import concourse.mybir as mybir  # noqa: E402
import concourse.tile as tile  # noqa: E402
from concourse import bacc  # noqa: E402
from concourse.bass import ts, ds  # noqa: E402
from concourse.bass_utils import run_bass_kernel_spmd  # noqa: E402

B, S, D = 2, 2048, 1024
HEADS, HD = 16, 64
G = 4                      # head groups == cores per batch element
HPC = HEADS // G           # heads per core
NCOL = HPC * HD            # 256 projection cols per core
KCH = D // 128             # K chunks
MCH = S // 128             # sk chunks
TCH = S // 512             # sq 512-blocks
F32 = mybir.dt.float32
F32R = mybir.dt.float32r
BF16 = mybir.dt.bfloat16
AF = mybir.ActivationFunctionType

FAST = False
TRACE = False
TRACE_DIR = None
LAST_EXEC_NS = None
_CACHE = {}


def _build(fast):
    xt_dt = BF16 if fast else F32R
    ev_dt = BF16 if fast else F32R

    nc = bacc.Bacc("TRN2")
    xT_d = nc.dram_tensor("xT", (D, S), xt_dt, kind="ExternalInput")
    wqe_d = nc.dram_tensor("wqe", (D, 128), xt_dt, kind="ExternalInput")
    wqo_d = nc.dram_tensor("wqo", (D, 128), xt_dt, kind="ExternalInput")
    wke_d = nc.dram_tensor("wke", (D, 128), xt_dt, kind="ExternalInput")
    wko_d = nc.dram_tensor("wko", (D, 128), xt_dt, kind="ExternalInput")
    wv_d = nc.dram_tensor("wv", (D, NCOL), xt_dt, kind="ExternalInput")
    wo_d = nc.dram_tensor("wo", (NCOL, D), F32R, kind="ExternalInput")
    cos_d = nc.dram_tensor("cosT", (128, S), F32, kind="ExternalInput")
    sin_d = nc.dram_tensor("sinT", (128, S), F32, kind="ExternalInput")
    tri_d = nc.dram_tensor("tri", (128, 128), ev_dt, kind="ExternalInput")
    ones1_d = nc.dram_tensor("ones1", (1, 64), F32R, kind="ExternalInput")
    vones_d = nc.dram_tensor("vones", (128, MCH * HPC), ev_dt,
                             kind="ExternalInput")
    out_d = nc.dram_tensor("out", (S, D), F32, kind="ExternalOutput")

    def mmr(ps, lhsT, rhs, **kw):
        nc.tensor.matmul(ps, lhsT, rhs, **kw)

    with tile.TileContext(nc) as tc:
        with tc.tile_pool(name="pp", bufs=1) as pp, \
             tc.tile_pool(name="tmp3", bufs=3) as tmp3, \
             tc.tile_pool(name="tmp2", bufs=2) as tmp2:
            rqe = pp.tile([128, S], BF16)
            rqo = pp.tile([128, S], BF16)
            rke = pp.tile([128, S], BF16)
            rko = pp.tile([128, S], BF16)
            v_aug = pp.tile([128, MCH, HPC, HD + 1], ev_dt)
            tri_sb = pp.tile([128, 128], ev_dt)
            ones1 = pp.tile([1, 64], F32R)
            nc.sync.dma_start(ones1[:], ones1_d[:])
            nc.sync.dma_start(
                v_aug[:, :, :, HD],
                vones_d.rearrange("p (m h) -> p m h", m=MCH))
            nc.sync.dma_start(tri_sb[:], tri_d[:])

            with tc.tile_pool(name="pA", bufs=1) as pA, \
                 tc.tile_pool(name="psQ", bufs=8, space="PSUM") as psQ:
                w_sb = {}
                for nm, d_t, w in (("wqe", wqe_d, 128), ("wqo", wqo_d, 128),
                                   ("wke", wke_d, 128), ("wko", wko_d, 128),
                                   ("wv", wv_d, NCOL)):
                    t_sb = pA.tile([128, KCH, w], xt_dt, tag=nm)
                    nc.sync.dma_start(
                        t_sb[:], d_t.rearrange("(ko p) m -> p ko m", p=128))
                    w_sb[nm] = t_sb
                cos_sb = pA.tile([128, S], F32)
                sin_sb = pA.tile([128, S], F32)
                nc.sync.dma_start(cos_sb[:], cos_d[:])
                nc.sync.dma_start(sin_sb[:], sin_d[:])
                xt = pA.tile([128, KCH, S], xt_dt)
                for k in range(KCH):
                    nc.sync.dma_start(xt[:, k, :], xT_d[ts(k, 128), :])

                def rope_pair(e_ps, o_ps, re, ro, sl):
                    t1 = tmp3.tile([128, 512], F32, tag="ropet")
                    t2 = tmp3.tile([128, 512], F32, tag="ropeu")
                    nc.vector.tensor_mul(t1[:], e_ps[:], cos_sb[:, sl])
                    nc.vector.tensor_mul(t2[:], o_ps[:], sin_sb[:, sl])
                    nc.vector.tensor_sub(re[:, sl], t1[:], t2[:])
                    t3 = tmp3.tile([128, 512], F32, tag="ropet")
                    t4 = tmp3.tile([128, 512], F32, tag="ropeu")
                    nc.vector.tensor_mul(t3[:], e_ps[:], sin_sb[:, sl])
                    nc.vector.tensor_mul(t4[:], o_ps[:], cos_sb[:, sl])
                    nc.vector.tensor_add(ro[:, sl], t3[:], t4[:])

                for we, wod, re, ro in ((w_sb["wqe"], w_sb["wqo"], rqe, rqo),
                                        (w_sb["wke"], w_sb["wko"], rke, rko)):
                    e_pss = [psQ.tile([128, 512], F32, tag="ps",
                                      name=f"eps{n}")
                             for n in range(TCH)]
                    o_pss = [psQ.tile([128, 512], F32, tag="ps",
                                      name=f"ops{n}")
                             for n in range(TCH)]
                    for k in range(KCH):
                        for n in range(TCH):
                            mmr(e_pss[n][:], we[:, k, :], xt[:, k, ts(n, 512)],
                                start=(k == 0), stop=(k == KCH - 1))
                            mmr(o_pss[n][:], wod[:, k, :],
                                xt[:, k, ts(n, 512)],
                                start=(k == 0), stop=(k == KCH - 1))
                    for n in range(TCH):
                        rope_pair(e_pss[n], o_pss[n], re, ro, ts(n, 512))

                for m in range(MCH):
                    v_ps = psQ.tile([128, NCOL], F32, tag="ps")
                    for k in range(KCH):
                        mmr(v_ps[:], xt[:, k, ts(m, 128)], w_sb["wv"][:, k, :],
                            start=(k == 0), stop=(k == KCH - 1))
                    nc.vector.tensor_copy(
                        v_aug[:, m, :, 0:HD],
                        v_ps.rearrange("p (h d) -> p h d", h=HPC))

            with tc.tile_pool(name="pF", bufs=1) as pF, \
                 tc.tile_pool(name="psS", bufs=3, space="PSUM") as psS, \
                 tc.tile_pool(name="psC", bufs=2, space="PSUM") as psC:
                wo_sb = pF.tile([128, 2, D], F32R)
                nc.sync.dma_start(
                    wo_sb[:], wo_d.rearrange("(j p) n -> p j n", p=128))
                ct = pF.tile([128, 2, S], F32R)

                for t in range(TCH):
                    sq = ts(t, 512)
                    nch = 4 * t + 4
                    for pair in range(2):
                        hs = (2 * pair, 2 * pair + 1)
                        ctx_ps = {h: psC.tile([65, 512], F32, tag="ctx",
                                              name=f"ctx{h}")
                                  for h in hs}
                        for cb in range(nch // 2):
                            c0 = 2 * cb
                            for h in hs:
                                # scores for chunks c0, c0+1 side by side in
                                # one 2-bank psum tile; one exp covers both
                                sc = psS.tile([128, 1024], F32, tag="sc",
                                              name=f"sc{h}")
                                e_sb = tmp3.tile([128, 1024], ev_dt,
                                                 tag="exp")
                                for half in range(2):
                                    c = c0 + half
                                    diag = (c // 4 == t)
                                    off = 128 * (c % 4) if diag else 0
                                    col = slice(512 * half + off,
                                                512 * half + 512)
                                    sqs = ds(512 * t + off, 512 - off)
                                    nc.tensor.matmul(
                                        sc[:, col],
                                        rke[32 * h:32 * h + 32, ts(c, 128)],
                                        rqe[32 * h:32 * h + 32, sqs],
                                        start=True, stop=False,
                                        tile_position=(32 * h, 0))
                                    nc.tensor.matmul(
                                        sc[:, col],
                                        rko[32 * h:32 * h + 32, ts(c, 128)],
                                        rqo[32 * h:32 * h + 32, sqs],
                                        start=False, stop=True,
                                        tile_position=(32 * h, 0))
                                nc.scalar.activation(e_sb[:], sc[:], AF.Exp)
                                for half in range(2):
                                    c = c0 + half
                                    diag = (c // 4 == t)
                                    off = 128 * (c % 4) if diag else 0
                                    if diag:
                                        dcol = slice(512 * half + off,
                                                     512 * half + off + 128)
                                        nc.vector.tensor_mul(
                                            e_sb[:, dcol], e_sb[:, dcol],
                                            tri_sb[:])
                                    ecol = slice(512 * half + off,
                                                 512 * half + 512)
                                    mmr(ctx_ps[h][:, ds(off, 512 - off)],
                                        v_aug[:, c, h, :], e_sb[:, ecol],
                                        start=(c == 0), stop=(c == nch - 1))
                        for h in hs:
                            d_sb = tmp2.tile([1, 512], F32R, tag="dsb")
                            nc.vector.tensor_copy(d_sb[:], ctx_ps[h][64:65, :])
                            rb_ps = psS.tile([128, 512], F32, tag="sc")
                            mmr(rb_ps[0:64, :], ones1[:], d_sb[:],
                                start=True, stop=True)
                            rec = tmp2.tile([64, 512], F32, tag="rec")
                            nc.vector.reciprocal_approx_fast(
                                rec[:], rb_ps[0:64, :])
                            nc.vector.tensor_mul(
                                ct[64 * (h % 2):64 * (h % 2) + 64, h // 2,
                                   sq],
                                ctx_ps[h][0:64, :], rec[:])
                    for m4 in range(4):
                        m = 4 * t + m4
                        for j2 in range(2):
                            o_ps = psS.tile([128, 512], F32, tag="sc")
                            mmr(o_ps[:], ct[:, 0, ts(m, 128)],
                                wo_sb[:, 0, ts(j2, 512)],
                                start=True, stop=False)
                            mmr(o_ps[:], ct[:, 1, ts(m, 128)],
                                wo_sb[:, 1, ts(j2, 512)],
                                start=False, stop=True)
                            o_sb = tmp2.tile([128, 512], F32, tag="osb")
                            nc.vector.tensor_copy(o_sb[:], o_ps[:])
                            nc.sync.dma_start(
                                out_d[ts(m, 128), ts(j2, 512)], o_sb[:])
    nc.compile()
    return nc


def _host_tables():
    half = HD // 2
    inv_freq = (1.0 / (10000.0 ** (np.arange(half, dtype=np.float32) / half)))
    angles = (np.arange(S, dtype=np.float32)[:, None]
              * inv_freq[None, :].astype(np.float32))
    cosT = np.tile(np.cos(angles).T.astype(np.float32), (HPC, 1))
    sinT = np.tile(np.sin(angles).T.astype(np.float32), (HPC, 1))
    i_idx = np.arange(128)[:, None]
    j_idx = np.arange(128)[None, :]
    tri = (j_idx >= i_idx).astype(np.float32)
    return np.ascontiguousarray(cosT), np.ascontiguousarray(sinT), tri


def kernel(x, Wq, Wk, Wv, Wo):
    global LAST_EXEC_NS
    x = np.asarray(x, dtype=np.float32)
    Wq = np.asarray(Wq, dtype=np.float32)
    Wk = np.asarray(Wk, dtype=np.float32)
    Wv = np.asarray(Wv, dtype=np.float32)
    Wo = np.asarray(Wo, dtype=np.float32)

    key = ("nc", FAST)
    if key not in _CACHE:
        _CACHE[key] = _build(FAST)
    nc = _CACHE[key]
    cosT, sinT, tri = _host_tables()

    xt_np = ml_dtypes.bfloat16 if FAST else np.float32
    ev_np = ml_dtypes.bfloat16 if FAST else np.float32

    in_maps = []
    for c in range(8):
        b, g = c // 4, c % 4
        cols = slice(g * NCOL, (g + 1) * NCOL)
        wq_g = Wq[:, cols].reshape(D, HPC, HD // 2, 2)
        wk_g = Wk[:, cols].reshape(D, HPC, HD // 2, 2)
        in_maps.append({
            "xT": np.ascontiguousarray(x[b].T).astype(xt_np),
            "wqe": np.ascontiguousarray(
                wq_g[..., 0].reshape(D, 128) * 0.125).astype(xt_np),
            "wqo": np.ascontiguousarray(
                wq_g[..., 1].reshape(D, 128) * 0.125).astype(xt_np),
            "wke": np.ascontiguousarray(
                wk_g[..., 0].reshape(D, 128)).astype(xt_np),
            "wko": np.ascontiguousarray(
                wk_g[..., 1].reshape(D, 128)).astype(xt_np),
            "wv": np.ascontiguousarray(Wv[:, cols]).astype(xt_np),
            "wo": np.ascontiguousarray(Wo[cols, :]),
            "cosT": cosT,
            "sinT": sinT,
            "tri": tri.astype(ev_np),
            "ones1": np.ones((1, 64), dtype=np.float32),
            "vones": np.ones((128, MCH * HPC), dtype=ev_np),
        })

    kw = {}
    if TRACE and TRACE_DIR:
        os.makedirs(TRACE_DIR, exist_ok=True)
        kw["tmpdir"] = TRACE_DIR
    res = run_bass_kernel_spmd(nc, in_maps, core_ids=list(range(8)),
                               trace=TRACE, **kw)
    LAST_EXEC_NS = res.exec_time_ns
    parts = [res.results[c]["out"] for c in range(8)]
    out = np.empty((B, S, D), dtype=np.float32)
    for b in range(B):
        out[b] = parts[4 * b] + parts[4 * b + 1] + parts[4 * b + 2] \
            + parts[4 * b + 3]
    return out
